# revision 14
# baseline (speedup 1.0000x reference)
"""Multi-head self-attention (B=2, S=2048, E=1024, H=16, D=64) on 8 TRN2 cores.

Sharding: core c handles batch b = c // 4 and heads (c % 4)*4 .. +4.
Each core computes a partial output projection over its 4 heads' slice of the
residual; the host sums the 4 partials per batch (equivalent to the TP
all-reduce, done on host since outputs are disjoint per batch group).

All matmuls run in float32r (fp32 with the low 12 mantissa bits zeroed -
s1e8m11), which streams at 1 cycle/row on the PE for moving dims >= 256,
4x faster than plain fp32. Inputs are pre-rounded on the host; intermediate
producers write with f32r output dtype so the engines round on write.

Per-core device pipeline:
  1. qkvT = w_sliceT.T @ xT   -> [128, 6, 2048] feature-major (q/k/v pairs)
  2. per head: V' = transpose(vT) with a ones column (softmax denominator
     comes out of the PV matmul for free); scoresT = K-tile @ Q chunks;
     P = exp(scores/8) (no max subtraction - score range is bounded by the
     input distribution); causal masking via precomputed binary masks on the
     diagonal tiles; PV accumulation into PSUM.
  3. normalize with reciprocal + rank-1 PE broadcast, store eT.
  4. partial out = eT.T @ w0cT, DMA to HBM.
"""

import sys
import types

import numpy as np


def _ensure_ntff_hook():
    """Install the axon NTFF profiling hook if the image's antenv lacks it.

    The container boot (sitecustomize -> trn_boot.boot) tries to register the
    hook via ``antenv.axon_hooks``; that module is missing in this image, so
    tracing silently degrades. Recreate the module and install the same
    ctypes-based hook so run_bass_kernel_spmd(trace=True) yields exec times.
    """
    try:
        import antenv.axon_hooks  # noqa: F401

        return
    except ImportError:
        pass
    try:
        import antenv
    except ImportError:
        return
    mod = types.ModuleType("antenv.axon_hooks")
    state = {"hook": None}
    mod.set_axon_ntff_profile_hook = lambda h: state.__setitem__("hook", h)
    mod.get_axon_ntff_profile_hook = lambda: state["hook"]
    sys.modules["antenv.axon_hooks"] = mod
    antenv.axon_hooks = mod
    try:
        from trn_agent_boot.trn_boot import _ntff_profile_via_ctypes

        mod.set_axon_ntff_profile_hook(
            _ntff_profile_via_ctypes("/opt/axon/libaxon_pjrt.so")
        )
    except Exception:
        pass


_ensure_ntff_hook()

import concourse.bass as bass
import concourse.mybir as mybir
import concourse.tile as tile
from concourse import bacc
from concourse.bass_utils import run_bass_kernel_spmd

F32 = mybir.dt.float32
F32R = mybir.dt.float32r

B, S, E, H, D = 2, 2048, 1024, 16, 64
NCORES = 8
CPB = 4            # cores per batch
HPC = H // CPB     # heads per core = 4
T = S              # tokens per core (one batch)
P = 128
QCH = 512          # query-chunk (psum free dim)
NCH = T // QCH     # 4 query chunks
NKT = T // P       # 16 key tiles
KO = E // P        # 8 contraction subtiles for the projections
NFT = 3 * HPC * D // P   # 6 feature tiles (q01,q23,k01,k23,v01,v23)
SCALE = 1.0 / np.sqrt(D)


def build_nc() -> bass.Bass:
    nc = bacc.Bacc(None, target_bir_lowering=False)
    xT_d = nc.dram_tensor("xT", [E, T], F32R, kind="ExternalInput")
    wT_d = nc.dram_tensor("wT", [E, NFT * P], F32R, kind="ExternalInput")
    w0T_d = nc.dram_tensor("w0T", [P, HPC // 2, E], F32R, kind="ExternalInput")
    mask_d = nc.dram_tensor("mask", [P, 4, QCH], F32, kind="ExternalInput")
    bcm_d = nc.dram_tensor("bcm", [P, P], F32R, kind="ExternalInput")
    id_d = nc.dram_tensor("ident", [P, P], F32R, kind="ExternalInput")
    vones_d = nc.dram_tensor("vones", [P, NKT], F32R, kind="ExternalInput")
    vfill_d = nc.dram_tensor("vfill", [P, NKT, 64], F32R, kind="ExternalInput")
    out_d = nc.dram_tensor("out", [T, E], F32, kind="ExternalOutput")

    with tile.TileContext(nc) as tc, nc.allow_low_precision(
        reason="f32r (11-bit mantissa) feeds the PE matmuls by design"
    ):
        _body(tc, xT_d, wT_d, w0T_d, mask_d, bcm_d, id_d, vones_d, vfill_d, out_d)
    nc.finalize()
    return nc


def _body(tc, xT_d, wT_d, w0T_d, mask_d, bcm_d, id_d, vones_d, vfill_d, out_d):
    nc = tc.nc
    with (
        tc.tile_pool(name="singles", bufs=1) as singles,
        tc.tile_pool(name="xchunks", bufs=2) as xchunks,
        tc.tile_pool(name="vps", bufs=2) as vps,
        tc.tile_pool(name="ptiles", bufs=4) as ptiles,
        tc.tile_pool(name="rtiles", bufs=2) as rtiles,
        tc.tile_pool(name="otiles", bufs=3) as otiles,
        tc.tile_pool(name="ps_mm", bufs=2, space="PSUM") as ps_mm,
        tc.tile_pool(name="ps_s", bufs=2, space="PSUM") as ps_s,
        tc.tile_pool(name="ps_pv", bufs=2, space="PSUM") as ps_pv,
        tc.tile_pool(name="ps_t", bufs=1, space="PSUM") as ps_t,
        tc.tile_pool(name="ps_bc", bufs=1, space="PSUM") as ps_bc,
    ):
        # --- constants / weights in SBUF ---
        w_sb = singles.tile([P, KO, NFT * P], F32R)
        nc.sync.dma_start(w_sb[:], wT_d.rearrange("(ko p) f -> p ko f", p=P))
        w0_sb = singles.tile([P, HPC // 2, E], F32R)
        nc.sync.dma_start(w0_sb[:], w0T_d[:])
        mask_sb = singles.tile([P, 4, QCH], F32)
        nc.sync.dma_start(mask_sb[:], mask_d[:])
        bcm_sb = singles.tile([P, P], F32R)
        nc.sync.dma_start(bcm_sb[:], bcm_d[:])
        ident = singles.tile([P, P], F32R)
        nc.sync.dma_start(ident[:], id_d[:])

        qkvT = singles.tile([P, NFT, T], F32R)
        eT = singles.tile([P, HPC // 2, T], F32R)

        # --- phase 1: fused QKV projection, feature-major output ---
        for tcx in range(T // QCH):
            xc = xchunks.tile([P, KO, QCH], F32R)
            nc.sync.dma_start(
                xc[:],
                xT_d[:, tcx * QCH : (tcx + 1) * QCH].rearrange(
                    "(ko p) t -> p ko t", p=P
                ),
            )
            for ft in range(NFT):
                pq = ps_mm.tile([P, QCH], F32, tag="mm512")
                for ko in range(KO):
                    nc.tensor.matmul(
                        pq[:],
                        w_sb[:, ko, ft * P : (ft + 1) * P],
                        xc[:, ko, :],
                        start=(ko == 0),
                        stop=(ko == KO - 1),
                    )
                nc.vector.tensor_copy(
                    qkvT[:, ft, tcx * QCH : (tcx + 1) * QCH], pq[:]
                )

        # --- phase 2: attention per head ---
        for h in range(HPC):
            g, r = h // 2, h % 2          # pair index, position in pair
            po = 64 * r                   # partition offset of this head
            qt = qkvT[po : po + 64, g, :]
            kt = qkvT[po : po + 64, 2 + g, :]
            vt = qkvT[po : po + 64, 4 + g, :]

            # V' tile: even head [128, 16, 65]: V in cols 0:64, ones col 64;
            # odd head [128, 16, 128]: ones col 32, zeros elsewhere in 0:64
            # (DMA'd frame), V in cols 64:128.
            if r == 0:
                vp = vps.tile([P, NKT, 65], F32R, tag="vpe")
                nc.sync.dma_start(vp[:, :, 64:65], vones_d[:, :, None])
                ones_col, v_cols = 64, 0
            else:
                vp = vps.tile([P, NKT, P], F32R, tag="vpo")
                nc.sync.dma_start(vp[:, :, 0:64], vfill_d[:])
                ones_col, v_cols = 32, 64
            for j in range(NKT):
                pt = ps_t.tile([P, 64], F32R, tag="tp")
                nc.tensor.transpose(
                    pt[:],
                    vt[:, j * P : (j + 1) * P],
                    ident[po : po + 64, po : po + 64],
                )
                nc.vector.tensor_copy(
                    vp[:, j, v_cols : v_cols + 64], pt[:].bitcast(F32)
                )

            for c in range(NCH):
                sq0 = c * QCH
                pv = ps_pv.tile([P, QCH], F32, tag="pv")
                jmax = 4 * (c + 1)
                for j in range(jmax):
                    sps = ps_s.tile([P, QCH], F32, tag="s")
                    nc.tensor.matmul(
                        sps[:],
                        kt[:, j * P : (j + 1) * P],
                        qt[:, sq0 : sq0 + QCH],
                        start=True,
                        stop=True,
                    )
                    p_t = ptiles.tile([P, QCH], F32R, tag="p")
                    nc.scalar.activation(
                        p_t[:], sps[:], mybir.ActivationFunctionType.Exp,
                        scale=float(SCALE),
                    )
                    if j >= 4 * c:
                        nc.vector.tensor_mul(
                            p_t[:], p_t[:].bitcast(F32), mask_sb[:, j - 4 * c, :]
                        )
                    nc.tensor.matmul(
                        pv[: vp.shape[2], :],
                        vp[:, j, :],
                        p_t[:],
                        start=(j == 0),
                        stop=(j == jmax - 1),
                    )
                # pv partitions: even head: e rows 0:64, denom row 64
                #                odd head:  e rows 64:128, denom row 32
                dp = ones_col
                rt = rtiles.tile([P, QCH], F32R, tag="r")
                nc.vector.reciprocal(rt[dp : dp + 1, :], pv[dp : dp + 1, :])
                bc = ps_bc.tile([P, QCH], F32, tag="bc")
                nc.tensor.matmul(
                    bc[:],
                    bcm_sb[dp : dp + 1, :],
                    rt[dp : dp + 1, :],
                    start=True,
                    stop=True,
                )
                rbc = rtiles.tile([P, QCH], F32, tag="rbc")
                nc.vector.tensor_copy(
                    rbc[po : po + 64, :], bc[po : po + 64, :]
                )
                nc.vector.tensor_mul(
                    eT[po : po + 64, g, sq0 : sq0 + QCH],
                    pv[po : po + 64, :],
                    rbc[po : po + 64, :],
                )

        # --- phase 3: partial output projection ---
        for tt in range(T // P):
            for oc in range(E // QCH):
                op = ps_mm.tile([P, QCH], F32, tag="mm512")
                for g in range(HPC // 2):
                    nc.tensor.matmul(
                        op[:],
                        eT[:, g, tt * P : (tt + 1) * P],
                        w0_sb[:, g, oc * QCH : (oc + 1) * QCH],
                        start=(g == 0),
                        stop=(g == HPC // 2 - 1),
                    )
                ot = otiles.tile([P, QCH], F32, tag="o")
                nc.vector.tensor_copy(ot[:], op[:])
                nc.sync.dma_start(
                    out_d[tt * P : (tt + 1) * P, oc * QCH : (oc + 1) * QCH],
                    ot[:],
                )


def round_f32r(a: np.ndarray) -> np.ndarray:
    """Round fp32 to fp32r (11-bit mantissa, RNE) - what the PE consumes."""
    b = np.ascontiguousarray(a, dtype=np.float32).view(np.uint32)
    lsb = (b >> np.uint32(12)) & np.uint32(1)
    r = (b + np.uint32(0x7FF) + lsb) & np.uint32(0xFFFFF000)
    return r.view(np.float32)


def make_inputs(x: np.ndarray, w_qkv: np.ndarray, w0: np.ndarray):
    """Build the 8 per-core input dicts."""
    x = np.ascontiguousarray(np.asarray(x, dtype=np.float32)).reshape(B, S, E)
    w_qkv = np.ascontiguousarray(np.asarray(w_qkv, dtype=np.float32))
    w0 = np.ascontiguousarray(np.asarray(w0, dtype=np.float32))

    mask = np.zeros((P, 4, QCH), dtype=np.float32)
    f = np.arange(QCH)[None, :]
    p = np.arange(P)[:, None]
    for m in range(4):
        mask[:, m, :] = (f >= 128 * m + p).astype(np.float32)

    bcm = np.zeros((P, P), dtype=np.float32)
    bcm[64, 0:64] = 1.0   # even head: denom at partition 64 -> rows 0:64
    bcm[32, 64:128] = 1.0  # odd head: denom at partition 32 -> rows 64:128

    ident = np.eye(P, dtype=np.float32)
    vones = np.ones((P, NKT), dtype=np.float32)
    vfill = np.zeros((P, NKT, 64), dtype=np.float32)
    vfill[:, :, 32] = 1.0

    xT_b = [round_f32r(x[b].T) for b in range(B)]

    in_maps = []
    for c in range(NCORES):
        b = c // CPB
        hb = (c % CPB) * HPC  # first head of this core
        rows = []
        for sec in range(3):  # q, k, v
            for g_ in range(HPC // 2):
                r0 = sec * E + (hb + 2 * g_) * D
                rows.append(w_qkv[r0 : r0 + 2 * D])
        w_slice = np.concatenate(rows, axis=0)  # [768, 1024]
        wT = round_f32r(w_slice.T)              # [1024, 768]

        w0T = np.empty((P, HPC // 2, E), dtype=np.float32)
        for g_ in range(HPC // 2):
            cols = slice((hb + 2 * g_) * D, (hb + 2 * g_ + 2) * D)
            w0T[:, g_, :] = w0[:, cols].T
        w0T = round_f32r(w0T)
        in_maps.append(
            {
                "xT": xT_b[b],
                "wT": wT,
                "w0T": w0T,
                "mask": mask,
                "bcm": bcm,
                "ident": ident,
                "vones": vones,
                "vfill": vfill,
            }
        )
    return in_maps


_NC_CACHE = None


def kernel(x, w_qkv, w0, trace=False, trace_cores=None):
    global _NC_CACHE
    if _NC_CACHE is None:
        _NC_CACHE = build_nc()
    nc = _NC_CACHE
    in_maps = make_inputs(x, w_qkv, w0)
    res = run_bass_kernel_spmd(
        nc, in_maps, list(range(NCORES)), trace=trace, trace_cores=trace_cores
    )
    kernel.last_results = res
    outs = [res.results[c]["out"] for c in range(NCORES)]
    full = np.empty((B, S, E), dtype=np.float32)
    for b in range(B):
        full[b] = np.sum(outs[b * CPB : (b + 1) * CPB], axis=0)
    return full
